# revision 4
# baseline (speedup 1.0000x reference)
"""Trainium2 Bass kernel for a 2-layer LSTM decoder (nn_Decoder).

Reference computation (per step, 30 steps):
    L0: gates = W_ih0 @ x + W_hh0 @ h0 + b;  LSTM cell -> h0', c0'
    L1: gates = W_ih1 @ h0' + W_hh1 @ h1 + b; LSTM cell -> h1', c1'
    out = W_hp @ h1' + b_hp ;  x_next = out

Key structural transform: for t >= 1, x_t = W_hp @ h1_{t-1} + b_hp, so the
layer-0 input contribution folds into an effective weight on h1_{t-1}:
    W_eff = W_ih0 @ W_hp          (512 x 128)
    b0_eff = b_ih0 + b_hh0 + W_ih0 @ b_hp
This removes the serial x-path entirely; only t = 0 uses the observed x.

Layout: all state is kept on-chip as [H=128 partitions, B_local free].
Data-parallel over batch: 8192 / 8 cores = 1024 per core, processed as
2 tiles of 512 (PSUM bank size for fp32 matmul outputs).

Matmuls run as float32r (full fp32 data, 1 cycle/row at N >= 256 on the PE).
"""

import numpy as np

import concourse.bass as bass
import concourse.mybir as mybir
from concourse import bacc
from concourse.tile import TileContext
from concourse.bass_utils import run_bass_kernel_spmd

N_CORES = 8
B = 8192
BL = B // N_CORES  # 1024 per core
H = 128
PRED = 30
NT = 2              # batch tiles per core
TN = BL // NT       # 512, free dim per tile (= one PSUM bank of fp32)

F32 = mybir.dt.float32
F32R = mybir.dt.float32r

_COMPILED = {}


def _r(ap):
    """View an fp32 AP as float32r for full-rate PE matmuls."""
    return ap.bitcast(F32R)


def build_bass(total_steps=PRED):
    nc = bacc.Bacc("TRN2", target_bir_lowering=False, debug=False)

    # Per-core inputs
    d_h0 = nc.declare_dram_parameter("h0T", [H, BL], F32R, isOutput=False)
    d_h1 = nc.declare_dram_parameter("h1T", [H, BL], F32R, isOutput=False)
    d_c0 = nc.declare_dram_parameter("c0T", [H, BL], F32, isOutput=False)
    d_c1 = nc.declare_dram_parameter("c1T", [H, BL], F32, isOutput=False)
    d_x0 = nc.declare_dram_parameter("x0T", [3, BL], F32R, isOutput=False)
    # Replicated weights
    d_wx0 = nc.declare_dram_parameter("wx0", [3, 4 * H], F32R, isOutput=False)
    d_weff = nc.declare_dram_parameter("weff", [H, 4 * H], F32R, isOutput=False)
    d_whh0 = nc.declare_dram_parameter("whh0", [H, 4 * H], F32R, isOutput=False)
    d_wih1 = nc.declare_dram_parameter("wih1", [H, 4 * H], F32R, isOutput=False)
    d_whh1 = nc.declare_dram_parameter("whh1", [H, 4 * H], F32R, isOutput=False)
    d_whp = nc.declare_dram_parameter("whp", [H, 3], F32, isOutput=False)
    d_b0f = nc.declare_dram_parameter("b0f", [H, 4], F32, isOutput=False)
    d_b0e = nc.declare_dram_parameter("b0e", [H, 4], F32, isOutput=False)
    d_b1 = nc.declare_dram_parameter("b1", [H, 4], F32, isOutput=False)
    d_bhp = nc.declare_dram_parameter("bhp", [H, 24], F32, isOutput=False)
    # Output: [t, p, c*3+d] where batch b = c*128 + p  (host reorders)
    d_out = nc.declare_dram_parameter("preds", [PRED, H, 24], F32, isOutput=True)

    SIG = mybir.ActivationFunctionType.Sigmoid
    TANH = mybir.ActivationFunctionType.Tanh

    with TileContext(nc) as tc:
        with (
            tc.tile_pool(name="const", bufs=1) as cpool,
            tc.tile_pool(name="state", bufs=1) as spool,
            tc.tile_pool(name="work", bufs=3) as wpool,
            tc.tile_pool(name="psg_pool", bufs=6, space="PSUM") as ppool,
            tc.tile_pool(name="po_pool", bufs=2, space="PSUM") as opool,
        ):
            # --- load constants / weights ---
            wx0 = cpool.tile([3, 4 * H], F32R)
            nc.sync.dma_start(out=wx0[:], in_=d_wx0[:])
            weff = cpool.tile([H, 4 * H], F32R)
            nc.sync.dma_start(out=weff[:], in_=d_weff[:])
            whh0 = cpool.tile([H, 4 * H], F32R)
            nc.sync.dma_start(out=whh0[:], in_=d_whh0[:])
            wih1 = cpool.tile([H, 4 * H], F32R)
            nc.sync.dma_start(out=wih1[:], in_=d_wih1[:])
            whh1 = cpool.tile([H, 4 * H], F32R)
            nc.sync.dma_start(out=whh1[:], in_=d_whh1[:])
            whp = cpool.tile([H, 3], F32)
            nc.sync.dma_start(out=whp[:], in_=d_whp[:])
            b0f = cpool.tile([H, 4], F32)
            nc.sync.dma_start(out=b0f[:], in_=d_b0f[:])
            b0e = cpool.tile([H, 4], F32)
            nc.sync.dma_start(out=b0e[:], in_=d_b0e[:])
            b1 = cpool.tile([H, 4], F32)
            nc.sync.dma_start(out=b1[:], in_=d_b1[:])
            bhp = cpool.tile([H, 24], F32)
            nc.sync.dma_start(out=bhp[:], in_=d_bhp[:])
            x0 = cpool.tile([3, BL], F32R)
            nc.sync.dma_start(out=x0[:], in_=d_x0[:])

            # --- state (persistent, updated in place) ---
            h_0 = spool.tile([H, BL], F32R)
            nc.sync.dma_start(out=h_0[:], in_=d_h0[:])
            h_1 = spool.tile([H, BL], F32R)
            nc.sync.dma_start(out=h_1[:], in_=d_h1[:])
            c_0 = spool.tile([H, BL], F32)
            nc.sync.dma_start(out=c_0[:], in_=d_c0[:])
            c_1 = spool.tile([H, BL], F32)
            nc.sync.dma_start(out=c_1[:], in_=d_c1[:])

            layers = [
                # (in_weight, rec_weight, bias, c_state, h_state)
                (None, whh0, None, c_0, h_0),   # L0 in-side varies by t
                (wih1, whh1, b1, c_1, h_1),
            ]

            for tt in range(total_steps):
                t = tt % PRED
                for li in (0, 1):
                    _, wrec, bias, c_st, h_st = layers[li]
                    if li == 0:
                        bias = b0f if tt == 0 else b0e
                        win = wx0 if tt == 0 else weff
                        rhs_in = x0 if tt == 0 else h_1
                    else:
                        win = wih1
                        rhs_in = h_0
                    for nt in range(NT):
                        sl = slice(nt * TN, (nt + 1) * TN)
                        # gate pre-activations -> PSUM (i, f, g, o chunks)
                        psg = []
                        for j in range(4):
                            pg = ppool.tile([H, TN], F32, name=f"pg{j}", tag="pg")
                            js = slice(j * H, (j + 1) * H)
                            nc.tensor.matmul(
                                pg[:], win[:, js], rhs_in[:, sl],
                                start=True, stop=False,
                            )
                            nc.tensor.matmul(
                                pg[:], wrec[:, js], h_st[:, sl],
                                start=False, stop=True,
                            )
                            psg.append(pg)
                        # activations (bias fused via per-partition bias operand)
                        sig_i = wpool.tile([H, TN], F32)
                        nc.scalar.activation(sig_i[:], psg[0][:], SIG, bias=bias[:, 0:1])
                        sig_f = wpool.tile([H, TN], F32)
                        nc.scalar.activation(sig_f[:], psg[1][:], SIG, bias=bias[:, 1:2])
                        tg = wpool.tile([H, TN], F32)
                        nc.scalar.activation(tg[:], psg[2][:], TANH, bias=bias[:, 2:3])
                        sig_o = wpool.tile([H, TN], F32)
                        nc.scalar.activation(sig_o[:], psg[3][:], SIG, bias=bias[:, 3:4])
                        # cell math
                        u = wpool.tile([H, TN], F32)
                        nc.vector.tensor_mul(u[:], sig_i[:], tg[:])
                        v = wpool.tile([H, TN], F32)
                        nc.vector.tensor_mul(v[:], sig_f[:], c_st[:, sl])
                        nc.vector.tensor_add(c_st[:, sl], u[:], v[:])
                        tch = wpool.tile([H, TN], F32)
                        nc.scalar.activation(tch[:], c_st[:, sl], TANH)
                        nc.vector.tensor_mul(h_st[:, sl], sig_o[:], tch[:])

                # projection: out^T chunks [128b, 3] via lhsT = h_1 column chunks
                po = opool.tile([H, 24], F32)
                for c8 in range(8):
                    cs = slice(c8 * H, (c8 + 1) * H)
                    nc.tensor.matmul(
                        po[:, 3 * c8:3 * c8 + 3], h_1[:, cs].bitcast(F32), whp[:],
                        start=(c8 == 0), stop=(c8 == 7),
                    )
                out_stage = wpool.tile([H, 24], F32)
                nc.vector.tensor_add(out_stage[:], po[:], bhp[:])
                nc.sync.dma_start(out=d_out[t], in_=out_stage[:])

    nc.compile()
    return nc


def _get_compiled(total_steps=PRED):
    if total_steps not in _COMPILED:
        _COMPILED[total_steps] = build_bass(total_steps)
    return _COMPILED[total_steps]


def prep_inputs(obs_traj_rel, h0, c0, W_ih0, W_hh0, b_ih0, b_hh0,
                W_ih1, W_hh1, b_ih1, b_hh1, W_hp, b_hp):
    f = np.float32
    asc = np.ascontiguousarray

    W_eff = (W_ih0 @ W_hp).astype(f)          # [512, 128]
    b0_first = (b_ih0 + b_hh0).astype(f)
    b0_eff = (b_ih0 + b_hh0 + W_ih0 @ b_hp).astype(f)
    b1v = (b_ih1 + b_hh1).astype(f)

    shared = {
        "wx0": asc(W_ih0.T.astype(f)),        # [3, 512]
        "weff": asc(W_eff.T),                  # [128, 512]
        "whh0": asc(W_hh0.T.astype(f)),
        "wih1": asc(W_ih1.T.astype(f)),
        "whh1": asc(W_hh1.T.astype(f)),
        "whp": asc(W_hp.T.astype(f)),          # [128, 3]
        "b0f": asc(b0_first.reshape(4, H).T),  # [128, 4]
        "b0e": asc(b0_eff.reshape(4, H).T),
        "b1": asc(b1v.reshape(4, H).T),
        "bhp": asc(np.tile(b_hp.astype(f), (H, 8))),  # [128, 24]
    }
    assert shared["bhp"].shape == (H, 24)

    h0T = np.transpose(h0, (0, 2, 1))          # [2, 128, 8192]
    c0T = np.transpose(c0, (0, 2, 1))
    x0T = obs_traj_rel[-1].T                    # [3, 8192]

    in_maps = []
    for c in range(N_CORES):
        bsl = slice(c * BL, (c + 1) * BL)
        m = dict(shared)
        m["h0T"] = asc(h0T[0, :, bsl].astype(f))
        m["h1T"] = asc(h0T[1, :, bsl].astype(f))
        m["c0T"] = asc(c0T[0, :, bsl].astype(f))
        m["c1T"] = asc(c0T[1, :, bsl].astype(f))
        m["x0T"] = asc(x0T[:, bsl].astype(f))
        in_maps.append(m)
    return in_maps


def run(in_maps, total_steps=PRED, **kw):
    nc = _get_compiled(total_steps)
    return run_bass_kernel_spmd(nc, in_maps, list(range(N_CORES)), **kw)


def gather(res_results):
    # per-core preds [30, 128, 24] with b_local = c*128 + p, col = c*3 + d
    outs = []
    for i in range(N_CORES):
        o = res_results[i]["preds"]                      # [30, 128, 24]
        o = o.reshape(PRED, H, 8, 3).transpose(0, 2, 1, 3)  # [30, 8, 128, 3]
        outs.append(o.reshape(PRED, BL, 3))
    return np.concatenate(outs, axis=1)                   # [30, 8192, 3]


def kernel(**inputs):
    inputs = {k: np.asarray(v) for k, v in inputs.items()}
    in_maps = prep_inputs(**inputs)
    res = run(in_maps)
    return gather(res.results)
